# revision 1
# baseline (speedup 1.0000x reference)
"""Trainium2 Bass kernel for nn_DeforConv_71605694759687.

ResBlock(stride2, 64->128) + DCNv2 (modulated deformable conv) + BN + ReLU.

Sharding (8 cores): (batch b = core//4, H-quarter q = core%4); each core
computes 32 output rows of out[b] end-to-end locally (halo via recompute,
no collectives).

Deformable sampling is gather-free: bilinear sampling at (base + k + d),
|d| < 2, is expanded over a 5x5 window of static shifts j with tent
coefficients t_j = relu(1 - |d - j|) (exactly the bilinear weights;
self-pruning to zero outside the active 2x2 cell).  The mask*ty*tx
coefficient maps are partition-broadcast across the 64 channels of each
deform group via tiny K=2 selector matmuls on the PE, and the rhs of the
main einsum is the Hadamard product C_bcast * shifted-feat-view (DVE,
bf16), contracted on the PE over (k, sigma, d, c).
"""

import numpy as np
import ml_dtypes
from contextlib import ExitStack

import concourse.bass as bass
import concourse.tile as tile
from concourse import mybir, bacc
from concourse.bass_utils import run_bass_kernel_spmd

F32 = mybir.dt.float32
BF16 = mybir.dt.bfloat16
AL = mybir.AluOpType
AF = mybir.ActivationFunctionType

P = 128
EPS = 1e-5
Ci, Co, DG, Cg = 64, 128, 2, 64
H, W = 128, 128          # output spatial (after stride-2)
QROWS = 32               # output rows per core
JW = (-2, -1, 0, 1, 2)   # tent window per axis (exact for |offset| < 2)
NSIG = 25
FR, FC = 38, 134         # F_pad: rows h0-3..h0+34, cols x in [-3,130]
F1R, F1C = 40, 130       # feat1: rows h0-4..h0+35, cols [-1,128]
XR, XC = 81, 258         # x_pad: rows 2*h0-9..2*h0+71, cols [-1,256]
NCHUNK = 1024
NPC = 4


def _bf(x):
    return np.ascontiguousarray(x.astype(ml_dtypes.bfloat16))


def _f(x):
    return np.ascontiguousarray(np.asarray(x, dtype=np.float32))


def build_nc():
    nc = bacc.Bacc(None)

    d_x = nc.dram_tensor("x_shard", [Ci, XR, XC], F32, kind="ExternalInput")
    d_l1 = nc.dram_tensor("lhsT1", [Ci, 9, P], F32, kind="ExternalInput")
    d_l2 = nc.dram_tensor("lhsT2", [P, 9, P], F32, kind="ExternalInput")
    d_lsc = nc.dram_tensor("lhsT_sc", [Ci, P], F32, kind="ExternalInput")
    d_loff = nc.dram_tensor("lhsT_off", [P, 9, 54], F32, kind="ExternalInput")
    d_ldcn = nc.dram_tensor("lhsT_dcn", [P, 9, P], BF16, kind="ExternalInput")
    d_esel = nc.dram_tensor("e_sel", [P, 9, P], BF16, kind="ExternalInput")
    d_cst = nc.dram_tensor("consts", [P, 16], F32, kind="ExternalInput")
    d_bq = nc.dram_tensor("bias_q", [P, 3], F32, kind="ExternalInput")
    d_rm1 = nc.dram_tensor("rowmask1", [P, F1R], F32, kind="ExternalInput")
    d_rmf = nc.dram_tensor("rowmaskF", [P, FR], F32, kind="ExternalInput")
    d_out = nc.dram_tensor("out", [P, QROWS, W], F32, kind="ExternalOutput")

    with tile.TileContext(nc) as tc, ExitStack() as ctx:
        singles = ctx.enter_context(tc.tile_pool(name="singles", bufs=1))

        # ---- persistent SBUF ----
        fpadA = singles.tile([P, FR, FC], F32)      # col c <-> x-3
        fA = singles.tile([P, FR, FC], BF16)        # bf16, col c <-> x-3
        fB = singles.tile([P, FR, FC], BF16)        # bf16, col c <-> x-2
        ldcn = singles.tile([P, 9, P], BF16)
        esel = singles.tile([P, 9, P], BF16)
        cst = singles.tile([P, 16], F32)

        nc.sync.dma_start(out=ldcn[:], in_=d_ldcn[:])
        nc.sync.dma_start(out=esel[:], in_=d_esel[:])
        nc.sync.dma_start(out=cst[:], in_=d_cst[:])

        inv1, beta1 = cst[:, 0:1], cst[:, 1:2]
        inv2, beta2 = cst[:, 2:3], cst[:, 3:4]
        inv3, beta3 = cst[:, 4:5], cst[:, 5:6]

        nc.vector.memset(fpadA[:, :, 0:3], 0.0)
        nc.vector.memset(fpadA[:, :, FC - 3:FC], 0.0)

        # ================= Phase A: ResBlock =================
        with tc.tile_pool(name="ph_a", bufs=1) as pa, \
             tc.tile_pool(name="psum_a", bufs=2, space="PSUM") as psa:
            x_pad = pa.tile([Ci, XR, XC], F32)
            feat1 = pa.tile([P, F1R, F1C], F32)
            l1 = pa.tile([Ci, 9, P], F32)
            l2 = pa.tile([P, 9, P], F32)
            lsc = pa.tile([Ci, P], F32)
            rm1 = pa.tile([P, F1R], F32)
            rmf = pa.tile([P, FR], F32)
            for i in range(8):
                r0, r1 = (XR * i) // 8, (XR * (i + 1)) // 8
                nc.sync.dma_start(out=x_pad[:, r0:r1, :],
                                  in_=d_x[:, r0:r1, :])
            for t, dref in ((l1, d_l1), (l2, d_l2),
                            (lsc, d_lsc), (rm1, d_rm1), (rmf, d_rmf)):
                nc.sync.dma_start(out=t[:], in_=dref[:])

            nc.vector.memset(feat1[:, :, 0:1], 0.0)
            nc.vector.memset(feat1[:, :, F1C - 1:F1C], 0.0)

            # conv1 3x3 s2 + bn1 + relu -> feat1
            # feat1 row f1 <-> global h0-4+f1; reads x_pad rows 2*f1+ty,
            # cols 2*c+tx; writes cols 1..128
            for cki in range(10):
                r0 = cki * 4
                ps = psa.tile([P, 4, W], F32)
                for t in range(9):
                    ty, tx = t // 3, t % 3
                    rhs = x_pad[:, 2 * r0 + ty: 2 * r0 + ty + 7: 2,
                                tx: tx + 2 * W - 1: 2]
                    nc.tensor.matmul(ps[:], l1[:, t, :], rhs,
                                     start=(t == 0), stop=(t == 8))
                nc.scalar.activation(feat1[:, r0:r0 + 4, 1:1 + W], ps[:],
                                     AF.Relu, bias=beta1, scale=inv1)
            nc.vector.tensor_tensor(
                feat1[:], feat1[:],
                rm1[:, :, None].to_broadcast(feat1.shape), AL.mult)

            # conv2 3x3 s1 (+ folded shortcut) + bn + relu -> fpadA
            # fpad row f2 <-> global h0-3+f2; feat1 rows f2+ty cols c+tx;
            # shortcut x_pad rows 2*f2+3, cols 2*c+1; writes cols 3..130
            for cki in range(10):
                r0 = cki * 4
                nrow = min(4, FR - r0)
                ps = psa.tile([P, 4, W], F32, tag="ps2")
                for t in range(9):
                    ty, tx = t // 3, t % 3
                    rhs = feat1[:, r0 + ty: r0 + ty + nrow, tx: tx + W]
                    nc.tensor.matmul(ps[:, :nrow], l2[:, t, :], rhs,
                                     start=(t == 0), stop=False)
                rhs_sc = x_pad[:, 2 * r0 + 3: 2 * r0 + 2 + 2 * nrow: 2,
                               1: 2 * W: 2]
                nc.tensor.matmul(ps[:, :nrow], lsc[:], rhs_sc,
                                 start=False, stop=True)
                nc.scalar.activation(fpadA[:, r0:r0 + nrow, 3:3 + W],
                                     ps[:, :nrow], AF.Relu,
                                     bias=beta2, scale=inv2)
            nc.vector.tensor_tensor(
                fpadA[:], fpadA[:],
                rmf[:, :, None].to_broadcast(fpadA.shape), AL.mult)

        late = ctx.enter_context(tc.tile_pool(name="late", bufs=1))
        c_all = late.tile([P, NSIG, NCHUNK], BF16)  # rows pq*32+2k+d

        nc.vector.tensor_copy(out=fA[:], in_=fpadA[:])
        nc.vector.tensor_copy(out=fB[:, :, 0:FC - 1], in_=fpadA[:, :, 1:FC])
        nc.vector.memset(fB[:, :, FC - 1:FC], 0.0)

        # ================= Phase B: offsets -> coefficients =================
        with tc.tile_pool(name="ph_b", bufs=1) as pb, \
             tc.tile_pool(name="ph_b_tmp", bufs=2) as pbt, \
             tc.tile_pool(name="psum_b", bufs=2, space="PSUM") as psb:
            loff = pb.tile([P, 9, 54], F32)
            bq = pb.tile([P, 3], F32)
            q_t = pb.tile([P, 3, NCHUNK], F32)     # dy, dx, mm
            mask_t = pb.tile([P, NCHUNK], BF16)
            ty_t = pb.tile([P, 5, NCHUNK], BF16)
            tx_t = pb.tile([P, 5, NCHUNK], BF16)
            nc.sync.dma_start(out=loff[:], in_=d_loff[:])
            nc.sync.dma_start(out=bq[:], in_=d_bq[:])

            # offset conv om: rows quant*18+k*2+d; out rows h0..h0+31
            # out row r: fpad rows r+2+ty, cols c+2+tx
            for cki in range(8):
                r0 = cki * 4
                pq, c2 = cki // 2, cki % 2
                ps = psb.tile([54, 4, W], F32)
                for t in range(9):
                    ty, tx = t // 3, t % 3
                    rhs = fpadA[:, r0 + 2 + ty: r0 + 6 + ty, 2 + tx: 2 + tx + W]
                    nc.tensor.matmul(ps[:], loff[:, t, :], rhs,
                                     start=(t == 0), stop=(t == 8))
                om_sb = pbt.tile([54, 512], F32, tag="om_sb")
                nc.scalar.copy(om_sb[:], ps[:].rearrange("p a b -> p (a b)"))
                for q in range(3):
                    nc.gpsimd.dma_start(
                        out=q_t[pq * 32: pq * 32 + 18, q,
                                c2 * 512:(c2 + 1) * 512],
                        in_=om_sb[q * 18:(q + 1) * 18, :])

            nc.vector.tensor_tensor(
                q_t[:], q_t[:], bq[:, :, None].to_broadcast(q_t.shape),
                AL.add)
            nc.scalar.activation(mask_t[:], q_t[:, 2, :], AF.Sigmoid)
            # tents t_j = relu(1 - |d - j|)
            for ax, dst in ((0, ty_t), (1, tx_t)):
                for ji, j in enumerate(JW):
                    a = pbt.tile([P, NCHUNK], F32, tag="tent_tmp")
                    nc.scalar.activation(a[:], q_t[:, ax, :], AF.Abs,
                                         bias=cst[:, 8 + ji:9 + ji])
                    nc.vector.tensor_scalar(a[:], a[:], -1.0, 1.0,
                                            AL.mult, AL.add)
                    nc.vector.tensor_scalar(dst[:, ji, :], a[:], 0.0, None,
                                            AL.max)
            # C = mask * ty * tx  -> bf16
            for ji in range(5):
                a = pbt.tile([P, NCHUNK], BF16, tag="prod_tmp")
                nc.vector.tensor_tensor(a[:], mask_t[:], ty_t[:, ji, :],
                                        AL.mult)
                for xi in range(5):
                    nc.vector.tensor_tensor(c_all[:, ji * 5 + xi, :], a[:],
                                            tx_t[:, xi, :], AL.mult)

        # ================= Phase C: bcast + Hadamard + einsum =================
        # p-chunks processed in pairs; the two chunks' selector matmuls sit in
        # adjacent 32-row groups (tile_position) so the PE overlaps them.
        with tc.tile_pool(name="psum_cb", bufs=1, space="PSUM") as psum_cb, \
             tc.tile_pool(name="psum_out", bufs=1, space="PSUM") as psum_out, \
             tc.tile_pool(name="cbs", bufs=4) as cbs_pool, \
             tc.tile_pool(name="rhs", bufs=4) as rhs_pool, \
             tc.tile_pool(name="outs", bufs=2) as out_pool:
            for pp in range(2):
                pos = [psum_out.tile([P, NCHUNK], F32, name=f"po{h}", tag=f"po{h}")
                       for h in range(2)]
                nj = 0
                for k in range(9):
                    ky, kx = k // 3, k % 3
                    for ji, jy in enumerate(JW):
                        for xi, jx in enumerate(JW):
                            sig = ji * 5 + xi
                            sy, sx = ky - 1 + jy, kx - 1 + jx
                            for h in range(2):
                                pc = pp * 2 + h
                                yb = pc * 8
                                rowp = pc * 32
                                cb = psum_cb.tile([P, NCHUNK], F32,
                                                  name=f"cb{h}", tag=f"cb{h}")
                                for h5 in range(2):
                                    nc.tensor.matmul(
                                        cb[:, h5 * 512:(h5 + 1) * 512],
                                        esel[rowp: rowp + 18, k, :],
                                        c_all[rowp: rowp + 18, sig,
                                              h5 * 512:(h5 + 1) * 512],
                                        start=True, stop=True,
                                        tile_position=(rowp, 0))
                                cbs = cbs_pool.tile([P, NCHUNK], BF16)
                                nc.scalar.copy(cbs[:], cb[:])
                                if (3 + sx) % 2 == 0:
                                    fv = fA[:, yb + 3 + sy: yb + 11 + sy,
                                            3 + sx: 131 + sx]
                                else:
                                    fv = fB[:, yb + 3 + sy: yb + 11 + sy,
                                            2 + sx: 130 + sx]
                                rt = rhs_pool.tile([P, NCHUNK], BF16)
                                nc.vector.tensor_tensor(
                                    rt[:].rearrange("p (a b) -> p a b", a=8),
                                    cbs[:].rearrange("p (a b) -> p a b", a=8),
                                    fv, AL.mult)
                                for h5 in range(2):
                                    nc.tensor.matmul(
                                        pos[h][:, h5 * 512:(h5 + 1) * 512],
                                        ldcn[:, k, :],
                                        rt[:, h5 * 512:(h5 + 1) * 512],
                                        start=(nj == 0),
                                        stop=(nj == 9 * NSIG - 1))
                            nj += 1
                for h in range(2):
                    pc = pp * 2 + h
                    yb = pc * 8
                    ob = out_pool.tile([P, NCHUNK], F32)
                    nc.scalar.activation(ob[:], pos[h][:], AF.Relu,
                                         bias=beta3, scale=inv3)
                    nc.sync.dma_start(
                        out=d_out[:, yb: yb + 8, :],
                        in_=ob[:].rearrange("p (a b) -> p a b", a=8))

    nc.compile()
    return nc


_CACHE = {}


def _prep(inputs):
    f = {k: _f(v) for k, v in inputs.items()}
    inv1 = f['g1'] / np.sqrt(f['v1'] + EPS)
    beta1 = f['b1'] - f['m1'] * inv1
    inv2 = f['g2'] / np.sqrt(f['v2'] + EPS)
    beta2 = f['b2'] - f['m2'] * inv2
    invd = f['gd'] / np.sqrt(f['vd'] + EPS)
    betad = f['bd'] - f['md'] * invd
    inv3 = f['g3'] / np.sqrt(f['v3'] + EPS)
    beta3 = f['b3'] - f['m3'] * inv3

    lhsT1 = np.transpose(f['w1'], (1, 2, 3, 0)).reshape(Ci, 9, P)
    lhsT2 = np.transpose(f['w2'], (1, 2, 3, 0)).reshape(P, 9, P)
    wd = f['wd'][:, :, 0, 0] * (invd / inv2)[:, None]
    lhsT_sc = np.ascontiguousarray(wd.T)

    # offset conv rows: quant*18 + k*2 + d  <-  orig quant*18 + d*9 + k
    perm = np.zeros(54, dtype=np.int64)
    for quant in range(3):
        for kk in range(9):
            for dd in range(2):
                perm[quant * 18 + kk * 2 + dd] = quant * 18 + dd * 9 + kk
    ow = f['off_w'][perm]
    obias = f['off_b'][perm]
    lhsT_off = np.transpose(ow, (1, 2, 3, 0)).reshape(P, 9, 54)

    wr = f['dcn_w'].reshape(Co, DG, Cg, 9)
    lhsT_dcn = np.transpose(wr, (1, 2, 3, 0)).reshape(P, 9, Co)

    esel = np.zeros((P, 9, P), dtype=np.float32)
    for s in range(4):
        for kk in range(9):
            for dd in range(2):
                esel[32 * s + 2 * kk + dd, kk, dd * 64:(dd + 1) * 64] = 1.0

    cst = np.zeros((P, 16), dtype=np.float32)
    for ji, j in enumerate((-2, -1, 0, 1, 2)):
        cst[:, 8 + ji] = -float(j)
    cst[:, 0], cst[:, 1] = inv1, beta1
    cst[:, 2], cst[:, 3] = inv2, beta2 + betad
    cst[:, 4], cst[:, 5] = inv3, beta3 + inv3 * f['dcn_b']

    bias_q = np.zeros((P, 3), dtype=np.float32)
    for pq in range(4):
        for kk in range(9):
            for dd in range(2):
                r = pq * 32 + kk * 2 + dd
                for quant in range(3):
                    bias_q[r, quant] = obias[quant * 18 + kk * 2 + dd]

    return dict(
        lhsT1=_f(lhsT1), lhsT2=_f(lhsT2), lhsT_sc=_f(lhsT_sc),
        lhsT_off=_f(lhsT_off), lhsT_dcn=_bf(lhsT_dcn), e_sel=_bf(esel),
        consts=_f(cst), bias_q=_f(bias_q), x=f['x'])


def kernel(**inputs):
    cfg = _prep(inputs)
    x = cfg.pop('x')
    B = x.shape[0]

    if 'nc' not in _CACHE:
        _CACHE['nc'] = build_nc()
    nc = _CACHE['nc']

    in_maps = []
    for cid in range(8):
        b, q = cid // 4, cid % 4
        h0 = 32 * q
        xp = np.zeros((Ci, XR, XC), dtype=np.float32)
        r_lo = 2 * h0 - 9
        s_lo, s_hi = max(r_lo, 0), min(2 * h0 + 72, 256)
        xp[:, s_lo - r_lo: s_hi - r_lo, 1:257] = x[b, :, s_lo:s_hi, :]
        rm1 = np.zeros((P, F1R), dtype=np.float32)
        for f1 in range(F1R):
            rm1[:, f1] = 1.0 if 0 <= h0 - 4 + f1 < H else 0.0
        rmf = np.zeros((P, FR), dtype=np.float32)
        for f2 in range(FR):
            rmf[:, f2] = 1.0 if 0 <= h0 - 3 + f2 < H else 0.0
        m = dict(cfg)
        m['x_shard'] = np.ascontiguousarray(xp)
        m['rowmask1'] = rm1
        m['rowmaskF'] = rmf
        in_maps.append(m)

    res = run_bass_kernel_spmd(nc, in_maps, core_ids=list(range(8)))
    out = np.zeros((B, Co, H, W), dtype=np.float32)
    for cid in range(8):
        b, q = cid // 4, cid % 4
        out[b, :, 32 * q:32 * q + 32, :] = res.results[cid]['out']
    return out



# revision 8
# speedup vs baseline: 2.5617x; 2.5617x over previous
"""Trainium2 Bass kernel for nn_DeforConv_71605694759687.

ResBlock(stride2, 64->128) + DCNv2 (modulated deformable conv) + BN + ReLU.

Sharding (8 cores): (batch b = core//4, H-quarter q = core%4); each core
computes 32 output rows of out[b] end-to-end locally (halo via recompute,
no collectives).

DCNv2 is computed by TRUE bilinear gather: the Pool engine's ap_gather
fetches (x0, x0+1) feature pairs (d=2 on an x-duplicated fp16 field) at
per-pixel integer corner positions, for the two y corners of each of the
9 taps.  Corner weights mask*(1-fy)(1-fx) etc. are partition-broadcast
across the 64 channels of each deform group via tiny selector matmuls on
the PE, Hadamard-multiplied with the gathered pairs on the DVE, and the
9 taps x 4 corners = 36 terms are contracted on the PE (fp16).

Gather pixel order (per 2048-px pass): j = c*16 + r_loc*2 + qq so the
int16 index wrap (partition j%16) lowers to plain strided DMAs; the
permutation is absorbed by AP views and a host-side reshape.
"""

import numpy as np
import ml_dtypes
from contextlib import ExitStack

import concourse.bass as bass
import concourse.tile as tile
from concourse import mybir, bacc
from concourse.bass_utils import run_bass_kernel_spmd

F32 = mybir.dt.float32
F16 = mybir.dt.float16
I16 = mybir.dt.int16
AL = mybir.AluOpType
AF = mybir.ActivationFunctionType

P = 128
EPS = 1e-5
Ci, Co, DG, Cg = 64, 128, 2, 64
H, W = 128, 128          # output spatial (after stride-2)
QROWS = 32               # output rows per core
FR, FC = 38, 134         # F field: rows h0-3..h0+34, cols x in [-3,130]
F1R, F1C = 40, 130       # feat1: rows h0-4..h0+35, cols [-1,128]
XR, XC = 81, 258         # x_pad: rows 2*h0-9..2*h0+71, cols [-1,256]
NCHUNK = 1024
FLAT = FR * FC           # 5092
PASSW = 16 * FC          # 2144: flat elems per pass row-base
INW = 21 * FC + 132 + 1  # 2947: gather in-window elems per pass


def _h(x):
    return np.ascontiguousarray(np.asarray(x, dtype=np.float32).astype(np.float16))


def _f(x):
    return np.ascontiguousarray(np.asarray(x, dtype=np.float32))


def build_nc():
    nc = bacc.Bacc(None)

    d_x = nc.dram_tensor("x_shard", [Ci, XR, XC], F16, kind="ExternalInput")
    d_l1 = nc.dram_tensor("lhsT1", [Ci, 9, P], F16, kind="ExternalInput")
    d_l2 = nc.dram_tensor("lhsT2", [P, 9, P], F16, kind="ExternalInput")
    d_lsc = nc.dram_tensor("lhsT_sc", [Ci, P], F16, kind="ExternalInput")
    d_loff = nc.dram_tensor("lhsT_off", [P, 9, 54], F16, kind="ExternalInput")
    d_ldcn = nc.dram_tensor("lhsT_dcn", [P, 9, P], F16, kind="ExternalInput")
    d_esel = nc.dram_tensor("e_sel", [P, 9, P], F16, kind="ExternalInput")
    d_cst = nc.dram_tensor("consts", [P, 8], F32, kind="ExternalInput")
    d_bq = nc.dram_tensor("bias_q", [P, 3], F32, kind="ExternalInput")
    d_by = nc.dram_tensor("base_y", [P, NCHUNK], F32, kind="ExternalInput")
    d_bx = nc.dram_tensor("base_x", [P, NCHUNK], F32, kind="ExternalInput")
    d_rm1 = nc.dram_tensor("rowmask1", [P, F1R], F32, kind="ExternalInput")
    d_rmf = nc.dram_tensor("rowmaskF", [P, FR], F32, kind="ExternalInput")
    # out layout: (pass, qq, m=c*8+r_loc) -> host re-maps
    d_out = nc.dram_tensor("out", [P, 2, 2, NCHUNK], F32, kind="ExternalOutput")
    d_scr = nc.dram_tensor("idx_scratch", [P, NCHUNK], I16, kind="Internal")

    with tile.TileContext(nc) as tc, ExitStack() as ctx:
        singles = ctx.enter_context(tc.tile_pool(name="singles", bufs=1))

        # ---- persistent SBUF ----
        fd = singles.tile([P, FLAT, 2], F16)     # x-pair duplicated F field
        ldcn = singles.tile([P, 9, P], F16)
        esel = singles.tile([P, 9, P], F16)
        cst = singles.tile([P, 8], F32)
        cw = singles.tile([P, 2, NCHUNK, 2], F16)   # corner wts (yc,px,xc)
        idxw = singles.tile([P, 2, 9, 256], I16)    # wrapped gather idx

        nc.sync.dma_start(out=ldcn[:], in_=d_ldcn[:])
        nc.sync.dma_start(out=esel[:], in_=d_esel[:])
        nc.sync.dma_start(out=cst[:], in_=d_cst[:])

        inv1, beta1 = cst[:, 0:1], cst[:, 1:2]
        inv2, beta2 = cst[:, 2:3], cst[:, 3:4]
        inv3, beta3 = cst[:, 4:5], cst[:, 5:6]

        fd4 = fd[:].rearrange("p (r c) e -> p r c e", r=FR)
        # zero the x-halo columns (cols 0..2 and 131..133 of every row)
        nc.vector.memset(fd4[:, :, 0:3, :], 0.0)
        nc.vector.memset(fd4[:, :, FC - 3:FC, :], 0.0)

        # ================= Phase A: ResBlock =================
        with tc.tile_pool(name="ph_a", bufs=1) as pa, \
             tc.tile_pool(name="psum_a", bufs=2, space="PSUM") as psa:
            x_pad = pa.tile([Ci, XR, XC], F16)
            feat1 = pa.tile([P, F1R, F1C], F16)
            l1 = pa.tile([Ci, 9, P], F16)
            l2 = pa.tile([P, 9, P], F16)
            lsc = pa.tile([Ci, P], F16)
            rm1 = pa.tile([P, F1R], F32)
            rmf = pa.tile([P, FR], F32)
            for i in range(8):
                r0, r1 = (XR * i) // 8, (XR * (i + 1)) // 8
                nc.sync.dma_start(out=x_pad[:, r0:r1, :],
                                  in_=d_x[:, r0:r1, :])
            for t, dref in ((l1, d_l1), (l2, d_l2),
                            (lsc, d_lsc), (rm1, d_rm1), (rmf, d_rmf)):
                nc.sync.dma_start(out=t[:], in_=dref[:])

            nc.vector.memset(feat1[:, :, 0:1], 0.0)
            nc.vector.memset(feat1[:, :, F1C - 1:F1C], 0.0)

            # conv1 3x3 s2 + bn1 + relu -> feat1
            for cki in range(10):
                r0 = cki * 4
                ps = psa.tile([P, 4, W], F32)
                for t in range(9):
                    ty, tx = t // 3, t % 3
                    rhs = x_pad[:, 2 * r0 + ty: 2 * r0 + ty + 7: 2,
                                tx: tx + 2 * W - 1: 2]
                    nc.tensor.matmul(ps[:], l1[:, t, :], rhs,
                                     start=(t == 0), stop=(t == 8))
                nc.scalar.activation(feat1[:, r0:r0 + 4, 1:1 + W], ps[:],
                                     AF.Relu, bias=beta1, scale=inv1)
            nc.vector.tensor_tensor(
                feat1[:], feat1[:],
                rm1[:, :, None].to_broadcast(feat1.shape), AL.mult)

            # conv2 3x3 s1 (+ folded shortcut) + bn + relu -> fd[:, :, 0]
            for cki in range(10):
                r0 = cki * 4
                nrow = min(4, FR - r0)
                ps = psa.tile([P, 4, W], F32, tag="ps2")
                for t in range(9):
                    ty, tx = t // 3, t % 3
                    rhs = feat1[:, r0 + ty: r0 + ty + nrow, tx: tx + W]
                    nc.tensor.matmul(ps[:, :nrow], l2[:, t, :], rhs,
                                     start=(t == 0), stop=False)
                rhs_sc = x_pad[:, 2 * r0 + 3: 2 * r0 + 2 + 2 * nrow: 2,
                               1: 2 * W: 2]
                nc.tensor.matmul(ps[:, :nrow], lsc[:], rhs_sc,
                                 start=False, stop=True)
                nc.scalar.activation(fd4[:, r0:r0 + nrow, 3:3 + W, 0],
                                     ps[:, :nrow], AF.Relu,
                                     bias=beta2, scale=inv2)
            nc.vector.tensor_tensor(
                fd4[:, :, :, 0], fd4[:, :, :, 0],
                rmf[:, :, None].to_broadcast([P, FR, FC]), AL.mult)
            # duplicate: fd[p, i, 1] = fd[p, i+1, 0]
            nc.vector.tensor_copy(out=fd[:, 0:FLAT - 1, 1],
                                  in_=fd[:, 1:FLAT, 0])
            nc.vector.memset(fd[:, FLAT - 1:FLAT, 1], 0.0)

        # ================= Phase B: offsets -> weights + indices ============
        with tc.tile_pool(name="ph_b", bufs=1) as pb, \
             tc.tile_pool(name="ph_b_tmp", bufs=2) as pbt, \
             tc.tile_pool(name="psum_b", bufs=2, space="PSUM") as psb:
            loff = pb.tile([P, 9, 54], F16)
            bq = pb.tile([P, 3], F32)
            basey = pb.tile([P, NCHUNK], F32)
            basex = pb.tile([P, NCHUNK], F32)
            q_t = pb.tile([P, 3, NCHUNK], F32)     # dy, dx, mm
            m_t = pb.tile([P, NCHUNK], F16)
            yf = pb.tile([P, NCHUNK], F32)
            xf = pb.tile([P, NCHUNK], F32)
            y0i = pb.tile([P, NCHUNK], I16)
            x0i = pb.tile([P, NCHUNK], I16)
            y0f = pb.tile([P, NCHUNK], F32)
            x0f = pb.tile([P, NCHUNK], F32)
            fy = pb.tile([P, NCHUNK], F16)
            fx = pb.tile([P, NCHUNK], F16)
            u0 = pb.tile([P, NCHUNK], F16)
            u1 = pb.tile([P, NCHUNK], F16)
            idxf = pb.tile([P, NCHUNK], F32)
            idx16 = pb.tile([P, NCHUNK], I16)
            nc.sync.dma_start(out=loff[:], in_=d_loff[:])
            nc.sync.dma_start(out=bq[:], in_=d_bq[:])
            nc.sync.dma_start(out=basey[:], in_=d_by[:])
            nc.sync.dma_start(out=basex[:], in_=d_bx[:])

            # offset conv om: rows quant*18+k*2+d; out rows h0..h0+31
            for cki in range(8):
                r0 = cki * 4
                pq, c2 = cki // 2, cki % 2
                ps = psb.tile([54, 4, W], F32)
                for t in range(9):
                    ty, tx = t // 3, t % 3
                    rhs = fd4[:, r0 + 2 + ty: r0 + 6 + ty,
                              2 + tx: 2 + tx + W, 0]
                    nc.tensor.matmul(ps[:], loff[:, t, :], rhs,
                                     start=(t == 0), stop=(t == 8))
                om_sb = pbt.tile([54, 512], F32, tag="om_sb")
                nc.scalar.copy(om_sb[:], ps[:].rearrange("p a b -> p (a b)"))
                for q in range(3):
                    nc.gpsimd.dma_start(
                        out=q_t[pq * 32: pq * 32 + 18, q,
                                c2 * 512:(c2 + 1) * 512],
                        in_=om_sb[q * 18:(q + 1) * 18, :])

            nc.vector.tensor_tensor(
                q_t[:], q_t[:], bq[:, :, None].to_broadcast(q_t.shape),
                AL.add)
            nc.scalar.activation(m_t[:], q_t[:, 2, :], AF.Sigmoid)
            # sample coords (pass-relative row base baked into base_y)
            nc.vector.tensor_tensor(yf[:], q_t[:, 0, :], basey[:], AL.add)
            nc.vector.tensor_tensor(xf[:], q_t[:, 1, :], basex[:], AL.add)
            # floor via RNE(v - 0.5), clamped to the legal corner range
            nc.vector.tensor_scalar(y0i[:], yf[:], -0.5, None, AL.add)
            nc.vector.tensor_scalar(x0i[:], xf[:], -0.5, None, AL.add)
            nc.vector.tensor_scalar(y0i[:], y0i[:], 0, 20, AL.max, AL.min)
            nc.vector.tensor_scalar(x0i[:], x0i[:], 0, 132, AL.max, AL.min)
            nc.vector.tensor_copy(out=y0f[:], in_=y0i[:])
            nc.vector.tensor_copy(out=x0f[:], in_=x0i[:])
            # fractional parts, clamped to [0, 1]
            nc.vector.scalar_tensor_tensor(fy[:], y0f[:], -1.0, yf[:],
                                           AL.mult, AL.add)
            nc.vector.scalar_tensor_tensor(fx[:], x0f[:], -1.0, xf[:],
                                           AL.mult, AL.add)
            nc.vector.tensor_scalar(fy[:], fy[:], 0.0, 1.0, AL.max, AL.min)
            nc.vector.tensor_scalar(fx[:], fx[:], 0.0, 1.0, AL.max, AL.min)
            # corner weights (mask folded in): cw[:, yc, :, xc]
            nc.vector.tensor_tensor(u1[:], m_t[:], fy[:], AL.mult)
            nc.vector.tensor_tensor(u0[:], m_t[:], u1[:], AL.subtract)
            nc.vector.tensor_tensor(cw[:, 0, :, 1], u0[:], fx[:], AL.mult)
            nc.vector.tensor_tensor(cw[:, 0, :, 0], u0[:], cw[:, 0, :, 1],
                                    AL.subtract)
            nc.vector.tensor_tensor(cw[:, 1, :, 1], u1[:], fx[:], AL.mult)
            nc.vector.tensor_tensor(cw[:, 1, :, 0], u1[:], cw[:, 1, :, 1],
                                    AL.subtract)
            # flat gather index (pass-relative)
            nc.vector.scalar_tensor_tensor(idxf[:], y0f[:], float(FC),
                                           x0f[:], AL.mult, AL.add)
            nc.vector.tensor_copy(out=idx16[:], in_=idxf[:])

            # wrap via DRAM scratch (DRAM APs have no partition constraints):
            # idxw[16g + 2*r_loc + qq, pp, k, c] = idx16[row(qq), r_loc*128+c]
            # row = (2pp+qq)*32 + 2k+dd;  groups g<4 take dd=0, g>=4 dd=1
            nc.sync.dma_start(out=d_scr[:], in_=idx16[:])
            srcv = d_scr[:].rearrange("(a p) (r c) -> a p r c", a=4, r=8)
            for pp in range(2):
                for k in range(9):
                    for dd in range(2):
                        # src element order (r, qq, c) = dst partition
                        # s = 2r + qq ascending, free c
                        src = srcv[2 * pp: 2 * pp + 2, 2 * k + dd, :, :] \
                            .rearrange("q r c -> r q c")
                        for g in range(4):
                            nc.gpsimd.dma_start(
                                out=idxw[dd * 64 + g * 16:
                                         dd * 64 + g * 16 + 16, pp, k, 0:128],
                                in_=src)
            # second half: y1 = y0 + FC
            for pp in range(2):
                nc.vector.tensor_scalar(idxw[:, pp, :, 128:256],
                                        idxw[:, pp, :, 0:128],
                                        FC, None, AL.add)

        # ================= Phase C: gather + Hadamard + einsum ==============
        with tc.tile_pool(name="psum_wb", bufs=2, space="PSUM") as psum_wb, \
             tc.tile_pool(name="psum_out", bufs=1, space="PSUM") as psum_out, \
             tc.tile_pool(name="gb", bufs=2) as gb_pool, \
             tc.tile_pool(name="wbs", bufs=2) as wbs_pool, \
             tc.tile_pool(name="rts", bufs=2) as rt_pool, \
             tc.tile_pool(name="outs", bufs=2) as out_pool:
            for pp in range(2):
                pos = [psum_out.tile([P, NCHUNK], F32, name=f"pos{pp}_{qq}",
                                     tag=f"pos{qq}") for qq in range(2)]
                for k in range(9):
                    g = gb_pool.tile([P, 4096, 2], F16)
                    nc.gpsimd.ap_gather(
                        g[:], fd[:, pp * PASSW: pp * PASSW + INW, :],
                        idxw[:, pp, k, :], channels=P,
                        num_elems=INW, d=2, num_idxs=4096)
                    # wbs layout: (yc, c, r_loc, qq, xc)
                    wbs = wbs_pool.tile([P, 2, 128, 8, 2, 2], F16)
                    for yc in range(2):
                        for xc in range(2):
                            for qq in range(2):
                                rowp = (2 * pp + qq) * 32
                                wb = psum_wb.tile([P, NCHUNK], F32, tag="wb")
                                # rhs in (c, r_loc) order -> psum col c*8+r
                                rhs = cw[rowp: rowp + 18, yc, :, xc] \
                                    .rearrange("p (r c) -> p c r", r=8)
                                for h5 in range(2):
                                    nc.tensor.matmul(
                                        wb[:, h5 * 512:(h5 + 1) * 512],
                                        esel[rowp: rowp + 18, k, :],
                                        rhs[:, h5 * 64:(h5 + 1) * 64, :],
                                        start=True, stop=True,
                                        tile_position=(rowp, 0))
                                nc.scalar.copy(
                                    wbs[:, yc, :, :, qq, xc],
                                    wb[:].rearrange("p (c r) -> p c r", c=128))
                    rt = rt_pool.tile([P, 2, 128, 8, 2, 2], F16)
                    nc.vector.tensor_tensor(
                        rt[:].rearrange("p a b c d e -> p (a b c d e)"),
                        wbs[:].rearrange("p a b c d e -> p (a b c d e)"),
                        g[:].rearrange("p a b -> p (a b)"),
                        AL.mult)
                    for yc in range(2):
                        for xc in range(2):
                            for qq in range(2):
                                rhs = rt[:, yc, :, :, qq, xc]
                                for h5 in range(2):
                                    nc.tensor.matmul(
                                        pos[qq][:, h5 * 512:(h5 + 1) * 512],
                                        ldcn[:, k, :],
                                        rhs[:, h5 * 64:(h5 + 1) * 64, :],
                                        start=(k == 0 and yc == 0
                                               and xc == 0),
                                        stop=(k == 8 and yc == 1
                                              and xc == 1))
                ob = out_pool.tile([P, 2, NCHUNK], F32)
                for qq in range(2):
                    nc.scalar.activation(ob[:, qq, :], pos[qq][:], AF.Relu,
                                         bias=beta3, scale=inv3)
                nc.sync.dma_start(out=d_out[:, pp, :, :], in_=ob[:])

    nc.compile()
    return nc


_CACHE = {}


def _prep(inputs):
    f = {k: _f(v) for k, v in inputs.items()}
    inv1 = f['g1'] / np.sqrt(f['v1'] + EPS)
    beta1 = f['b1'] - f['m1'] * inv1
    inv2 = f['g2'] / np.sqrt(f['v2'] + EPS)
    beta2 = f['b2'] - f['m2'] * inv2
    invd = f['gd'] / np.sqrt(f['vd'] + EPS)
    betad = f['bd'] - f['md'] * invd
    inv3 = f['g3'] / np.sqrt(f['v3'] + EPS)
    beta3 = f['b3'] - f['m3'] * inv3

    lhsT1 = np.transpose(f['w1'], (1, 2, 3, 0)).reshape(Ci, 9, P)
    lhsT2 = np.transpose(f['w2'], (1, 2, 3, 0)).reshape(P, 9, P)
    wd = f['wd'][:, :, 0, 0] * (invd / inv2)[:, None]
    lhsT_sc = np.ascontiguousarray(wd.T)

    # offset conv rows: quant*18 + k*2 + d  <-  orig quant*18 + d*9 + k
    perm = np.zeros(54, dtype=np.int64)
    for quant in range(3):
        for kk in range(9):
            for dd in range(2):
                perm[quant * 18 + kk * 2 + dd] = quant * 18 + dd * 9 + kk
    ow = f['off_w'][perm]
    obias = f['off_b'][perm]
    lhsT_off = np.transpose(ow, (1, 2, 3, 0)).reshape(P, 9, 54)

    wr = f['dcn_w'].reshape(Co, DG, Cg, 9)
    lhsT_dcn = np.transpose(wr, (1, 2, 3, 0)).reshape(P, 9, Co)

    esel = np.zeros((P, 9, P), dtype=np.float32)
    for s in range(4):
        for kk in range(9):
            for dd in range(2):
                esel[32 * s + 2 * kk + dd, kk, dd * 64:(dd + 1) * 64] = 1.0

    cst = np.zeros((P, 8), dtype=np.float32)
    cst[:, 0], cst[:, 1] = inv1, beta1
    cst[:, 2], cst[:, 3] = inv2, beta2 + betad
    cst[:, 4], cst[:, 5] = inv3, beta3 + inv3 * f['dcn_b']

    bias_q = np.zeros((P, 3), dtype=np.float32)
    for pq in range(4):
        for kk in range(9):
            for dd in range(2):
                r = pq * 32 + kk * 2 + dd
                for quant in range(3):
                    bias_q[r, quant] = obias[quant * 18 + kk * 2 + dd]

    # coordinate base maps (pass-relative row base)
    base_y = np.zeros((P, NCHUNK), dtype=np.float32)
    base_x = np.zeros((P, NCHUNK), dtype=np.float32)
    px = np.arange(NCHUNK)
    for p in range(P):
        pq, t = p // 32, p % 32
        if t >= 18:
            continue
        kk, dd = t // 2, t % 2
        ky, kx = kk // 3, kk % 3
        base_y[p] = 8 * (pq % 2) + px // 128 + ky + 2
        base_x[p] = px % 128 + kx + 2

    return dict(
        lhsT1=_h(lhsT1), lhsT2=_h(lhsT2), lhsT_sc=_h(lhsT_sc),
        lhsT_off=_h(lhsT_off), lhsT_dcn=_h(lhsT_dcn), e_sel=_h(esel),
        consts=_f(cst), bias_q=_f(bias_q), base_y=base_y, base_x=base_x,
        x=f['x'])


def kernel(**inputs):
    cfg = _prep(inputs)
    x = cfg.pop('x')
    B = x.shape[0]

    if 'nc' not in _CACHE:
        _CACHE['nc'] = build_nc()
    nc = _CACHE['nc']

    in_maps = []
    for cid in range(8):
        b, q = cid // 4, cid % 4
        h0 = 32 * q
        xp = np.zeros((Ci, XR, XC), dtype=np.float16)
        r_lo = 2 * h0 - 9
        s_lo, s_hi = max(r_lo, 0), min(2 * h0 + 72, 256)
        xp[:, s_lo - r_lo: s_hi - r_lo, 1:257] = \
            x[b, :, s_lo:s_hi, :].astype(np.float16)
        rm1 = np.zeros((P, F1R), dtype=np.float32)
        for f1 in range(F1R):
            rm1[:, f1] = 1.0 if 0 <= h0 - 4 + f1 < H else 0.0
        rmf = np.zeros((P, FR), dtype=np.float32)
        for f2 in range(FR):
            rmf[:, f2] = 1.0 if 0 <= h0 - 3 + f2 < H else 0.0
        m = dict(cfg)
        m['x_shard'] = np.ascontiguousarray(xp)
        m['rowmask1'] = rm1
        m['rowmaskF'] = rmf
        in_maps.append(m)

    res = run_bass_kernel_spmd(nc, in_maps, core_ids=list(range(8)))
    out = np.zeros((B, Co, H, W), dtype=np.float32)
    for cid in range(8):
        b, q = cid // 4, cid % 4
        o = res.results[cid]['out']            # [P, pp, qq, m=c*8+r_loc]
        o = o.reshape(P, 2, 2, 128, 8)         # [P, pp, qq, c, r]
        o = np.transpose(o, (0, 1, 2, 4, 3))   # [P, pp, qq, r, c]
        out[b, :, 32 * q:32 * q + 32, :] = o.reshape(P, 32, 128)
    return out


# revision 11
# speedup vs baseline: 3.6209x; 1.4135x over previous
"""Trainium2 Bass kernel for nn_DeforConv_71605694759687.

ResBlock(stride2, 64->128) + DCNv2 (modulated deformable conv) + BN + ReLU.

Sharding (8 cores): (batch b = core//4, H-quarter q = core%4); each core
computes 32 output rows of out[b] end-to-end locally (halo via recompute,
no collectives).

DCNv2 is computed by TRUE bilinear gather: the Pool engine's ap_gather
fetches (x0, x0+1) feature pairs (d=2 on an x-duplicated fp16 field) at
per-pixel integer corner positions, for the two y corners of each of the
9 taps.  Corner weights mask*(1-fy)(1-fx) etc. are partition-broadcast
across the 64 channels of each deform group via tiny selector matmuls on
the PE, Hadamard-multiplied with the gathered pairs on the DVE, and the
9 taps x 4 corners = 36 terms are contracted on the PE (fp16).

Gather pixel order (per 2048-px pass): j = c*16 + r_loc*2 + qq so the
int16 index wrap (partition j%16) lowers to plain strided DMAs; the
permutation is absorbed by AP views and a host-side reshape.
"""

import numpy as np
import ml_dtypes
from contextlib import ExitStack

import concourse.bass as bass
import concourse.tile as tile
from concourse import mybir, bacc
from concourse.bass_utils import run_bass_kernel_spmd

F32 = mybir.dt.float32
F16 = mybir.dt.float16
I16 = mybir.dt.int16
AL = mybir.AluOpType
AF = mybir.ActivationFunctionType

P = 128
EPS = 1e-5
Ci, Co, DG, Cg = 64, 128, 2, 64
H, W = 128, 128          # output spatial (after stride-2)
QROWS = 32               # output rows per core
FR, FC = 38, 134         # F field: rows h0-3..h0+34, cols x in [-3,130]
F1R, F1C = 40, 130       # feat1: rows h0-4..h0+35, cols [-1,128]
XR, XC = 81, 258         # x_pad: rows 2*h0-9..2*h0+71, cols [-1,256]
NCHUNK = 1024
FLAT = FR * FC           # 5092
PASSW = 16 * FC          # 2144: flat elems per pass row-base
INW = 21 * FC + 132 + 1  # 2947: gather in-window elems per pass


def _h(x):
    return np.ascontiguousarray(np.asarray(x, dtype=np.float32).astype(np.float16))


def _f(x):
    return np.ascontiguousarray(np.asarray(x, dtype=np.float32))


def build_nc():
    nc = bacc.Bacc(None)

    d_x = nc.dram_tensor("x_shard", [Ci, XR, XC], F16, kind="ExternalInput")
    d_l1 = nc.dram_tensor("lhsT1", [Ci, 9, P], F16, kind="ExternalInput")
    d_l2 = nc.dram_tensor("lhsT2", [P, 9, P], F16, kind="ExternalInput")
    d_lsc = nc.dram_tensor("lhsT_sc", [Ci, P], F16, kind="ExternalInput")
    d_loff = nc.dram_tensor("lhsT_off", [P, 9, 54], F16, kind="ExternalInput")
    d_ldcn = nc.dram_tensor("lhsT_dcn", [P, 9, P], F16, kind="ExternalInput")
    d_esel = nc.dram_tensor("e_sel", [P, 9, P], F16, kind="ExternalInput")
    d_cst = nc.dram_tensor("consts", [P, 8], F32, kind="ExternalInput")
    d_bq = nc.dram_tensor("bias_q", [P, 3], F32, kind="ExternalInput")
    d_by = nc.dram_tensor("base_y", [P, NCHUNK], F32, kind="ExternalInput")
    d_bx = nc.dram_tensor("base_x", [P, NCHUNK], F32, kind="ExternalInput")
    d_rm1 = nc.dram_tensor("rowmask1", [P, F1R], F32, kind="ExternalInput")
    d_rmf = nc.dram_tensor("rowmaskF", [P, FR], F32, kind="ExternalInput")
    # out layout: (pass, qq, m=c*8+r_loc) -> host re-maps
    d_out = nc.dram_tensor("out", [P, 2, 2, NCHUNK], F32, kind="ExternalOutput")
    d_scr = nc.dram_tensor("idx_scratch", [P, NCHUNK], I16, kind="Internal")

    with tile.TileContext(nc) as tc, ExitStack() as ctx:
        singles = ctx.enter_context(tc.tile_pool(name="singles", bufs=1))

        # ---- persistent SBUF ----
        fd = singles.tile([P, FLAT, 2], F16)     # x-pair duplicated F field
        ldcn = singles.tile([P, 9, P], F16)
        esel = singles.tile([P, 9, P], F16)
        cst = singles.tile([P, 8], F32)
        cw = singles.tile([P, 2, NCHUNK, 2], F16)   # corner wts (yc,px,xc)
        idxw = singles.tile([P, 2, 9, 256], I16)    # wrapped gather idx

        nc.sync.dma_start(out=ldcn[:], in_=d_ldcn[:])
        nc.sync.dma_start(out=esel[:], in_=d_esel[:])
        nc.sync.dma_start(out=cst[:], in_=d_cst[:])

        inv1, beta1 = cst[:, 0:1], cst[:, 1:2]
        inv2, beta2 = cst[:, 2:3], cst[:, 3:4]
        inv3, beta3 = cst[:, 4:5], cst[:, 5:6]

        fd4 = fd[:].rearrange("p (r c) e -> p r c e", r=FR)
        # zero the x-halo columns (cols 0..2 and 131..133 of every row)
        nc.vector.memset(fd4[:, :, 0:3, :], 0.0)
        nc.vector.memset(fd4[:, :, FC - 3:FC, :], 0.0)

        # ================= Phase A: ResBlock =================
        with tc.tile_pool(name="ph_a", bufs=1) as pa, \
             tc.tile_pool(name="psum_a", bufs=2, space="PSUM") as psa:
            x_pad = pa.tile([Ci, XR, XC], F16)
            feat1 = pa.tile([P, F1R, F1C], F16)
            l1 = pa.tile([Ci, 9, P], F16)
            l2 = pa.tile([P, 9, P], F16)
            lsc = pa.tile([Ci, P], F16)
            rm1 = pa.tile([P, F1R], F32)
            rmf = pa.tile([P, FR], F32)
            for i in range(8):
                r0, r1 = (XR * i) // 8, (XR * (i + 1)) // 8
                nc.sync.dma_start(out=x_pad[:, r0:r1, :],
                                  in_=d_x[:, r0:r1, :])
            for t, dref in ((l1, d_l1), (l2, d_l2),
                            (lsc, d_lsc), (rm1, d_rm1), (rmf, d_rmf)):
                nc.sync.dma_start(out=t[:], in_=dref[:])

            nc.vector.memset(feat1[:, :, 0:1], 0.0)
            nc.vector.memset(feat1[:, :, F1C - 1:F1C], 0.0)

            # conv1 3x3 s2 + bn1 + relu -> feat1
            for cki in range(10):
                r0 = cki * 4
                ps = psa.tile([P, 4, W], F32)
                for t in range(9):
                    ty, tx = t // 3, t % 3
                    rhs = x_pad[:, 2 * r0 + ty: 2 * r0 + ty + 7: 2,
                                tx: tx + 2 * W - 1: 2]
                    nc.tensor.matmul(ps[:], l1[:, t, :], rhs,
                                     start=(t == 0), stop=(t == 8))
                nc.scalar.activation(feat1[:, r0:r0 + 4, 1:1 + W], ps[:],
                                     AF.Relu, bias=beta1, scale=inv1)
            nc.vector.tensor_tensor(
                feat1[:], feat1[:],
                rm1[:, :, None].to_broadcast(feat1.shape), AL.mult)

            # conv2 3x3 s1 (+ folded shortcut) + bn + relu -> fd[:, :, 0]
            for cki in range(10):
                r0 = cki * 4
                nrow = min(4, FR - r0)
                ps = psa.tile([P, 4, W], F32, tag="ps2")
                for t in range(9):
                    ty, tx = t // 3, t % 3
                    rhs = feat1[:, r0 + ty: r0 + ty + nrow, tx: tx + W]
                    nc.tensor.matmul(ps[:, :nrow], l2[:, t, :], rhs,
                                     start=(t == 0), stop=False)
                rhs_sc = x_pad[:, 2 * r0 + 3: 2 * r0 + 2 + 2 * nrow: 2,
                               1: 2 * W: 2]
                nc.tensor.matmul(ps[:, :nrow], lsc[:], rhs_sc,
                                 start=False, stop=True)
                nc.scalar.activation(fd4[:, r0:r0 + nrow, 3:3 + W, 0],
                                     ps[:, :nrow], AF.Relu,
                                     bias=beta2, scale=inv2)
            nc.vector.tensor_tensor(
                fd4[:, :, :, 0], fd4[:, :, :, 0],
                rmf[:, :, None].to_broadcast([P, FR, FC]), AL.mult)
            # duplicate: fd[p, i, 1] = fd[p, i+1, 0]
            nc.vector.tensor_copy(out=fd[:, 0:FLAT - 1, 1],
                                  in_=fd[:, 1:FLAT, 0])
            nc.vector.memset(fd[:, FLAT - 1:FLAT, 1], 0.0)

        # ================= Phase B: offsets -> weights + indices ============
        with tc.tile_pool(name="ph_b", bufs=1) as pb, \
             tc.tile_pool(name="ph_b_tmp", bufs=2) as pbt, \
             tc.tile_pool(name="psum_b", bufs=2, space="PSUM") as psb:
            loff = pb.tile([P, 9, 54], F16)
            bq = pb.tile([P, 3], F32)
            basey = pb.tile([P, NCHUNK], F32)
            basex = pb.tile([P, NCHUNK], F32)
            q_t = pb.tile([P, 3, NCHUNK], F32)     # dy, dx, mm
            m_t = pb.tile([P, NCHUNK], F16)
            yf = pb.tile([P, NCHUNK], F32)
            xf = pb.tile([P, NCHUNK], F32)
            y0i = pb.tile([P, NCHUNK], I16)
            x0i = pb.tile([P, NCHUNK], I16)
            y0f = pb.tile([P, NCHUNK], F32)
            x0f = pb.tile([P, NCHUNK], F32)
            fy = pb.tile([P, NCHUNK], F16)
            fx = pb.tile([P, NCHUNK], F16)
            u0 = pb.tile([P, NCHUNK], F16)
            u1 = pb.tile([P, NCHUNK], F16)
            idxf = pb.tile([P, NCHUNK], F32)
            idx16 = pb.tile([P, NCHUNK], I16)
            nc.sync.dma_start(out=loff[:], in_=d_loff[:])
            nc.sync.dma_start(out=bq[:], in_=d_bq[:])
            nc.sync.dma_start(out=basey[:], in_=d_by[:])
            nc.sync.dma_start(out=basex[:], in_=d_bx[:])

            # offset conv om: rows quant*18+k*2+d; out rows h0..h0+31
            for cki in range(8):
                r0 = cki * 4
                pq, c2 = cki // 2, cki % 2
                ps = psb.tile([54, 4, W], F32)
                for t in range(9):
                    ty, tx = t // 3, t % 3
                    rhs = fd4[:, r0 + 2 + ty: r0 + 6 + ty,
                              2 + tx: 2 + tx + W, 0]
                    nc.tensor.matmul(ps[:], loff[:, t, :], rhs,
                                     start=(t == 0), stop=(t == 8))
                om_sb = pbt.tile([54, 512], F32, tag="om_sb")
                nc.scalar.copy(om_sb[:], ps[:].rearrange("p a b -> p (a b)"))
                for q in range(3):
                    nc.sync.dma_start(
                        out=q_t[pq * 32: pq * 32 + 18, q,
                                c2 * 512:(c2 + 1) * 512],
                        in_=om_sb[q * 18:(q + 1) * 18, :])

            nc.vector.tensor_tensor(
                q_t[:], q_t[:], bq[:, :, None].to_broadcast(q_t.shape),
                AL.add)
            nc.scalar.activation(m_t[:], q_t[:, 2, :], AF.Sigmoid)
            # sample coords (pass-relative row base baked into base_y)
            nc.vector.tensor_tensor(yf[:], q_t[:, 0, :], basey[:], AL.add)
            nc.vector.tensor_tensor(xf[:], q_t[:, 1, :], basex[:], AL.add)
            # floor via RNE(v - 0.5), clamped to the legal corner range
            nc.vector.tensor_scalar(y0i[:], yf[:], -0.5, None, AL.add)
            nc.vector.tensor_scalar(x0i[:], xf[:], -0.5, None, AL.add)
            nc.vector.tensor_scalar(y0i[:], y0i[:], 0, 20, AL.max, AL.min)
            nc.vector.tensor_scalar(x0i[:], x0i[:], 0, 132, AL.max, AL.min)
            nc.vector.tensor_copy(out=y0f[:], in_=y0i[:])
            nc.vector.tensor_copy(out=x0f[:], in_=x0i[:])
            # fractional parts, clamped to [0, 1]
            nc.vector.scalar_tensor_tensor(fy[:], y0f[:], -1.0, yf[:],
                                           AL.mult, AL.add)
            nc.vector.scalar_tensor_tensor(fx[:], x0f[:], -1.0, xf[:],
                                           AL.mult, AL.add)
            nc.vector.tensor_scalar(fy[:], fy[:], 0.0, 1.0, AL.max, AL.min)
            nc.vector.tensor_scalar(fx[:], fx[:], 0.0, 1.0, AL.max, AL.min)
            # corner weights (mask folded in): cw[:, yc, :, xc]
            nc.vector.tensor_tensor(u1[:], m_t[:], fy[:], AL.mult)
            nc.vector.tensor_tensor(u0[:], m_t[:], u1[:], AL.subtract)
            nc.vector.tensor_tensor(cw[:, 0, :, 1], u0[:], fx[:], AL.mult)
            nc.vector.tensor_tensor(cw[:, 0, :, 0], u0[:], cw[:, 0, :, 1],
                                    AL.subtract)
            nc.vector.tensor_tensor(cw[:, 1, :, 1], u1[:], fx[:], AL.mult)
            nc.vector.tensor_tensor(cw[:, 1, :, 0], u1[:], cw[:, 1, :, 1],
                                    AL.subtract)
            # flat gather index (pass-relative)
            nc.vector.scalar_tensor_tensor(idxf[:], y0f[:], float(FC),
                                           x0f[:], AL.mult, AL.add)
            nc.vector.tensor_copy(out=idx16[:], in_=idxf[:])

            # wrap via DRAM scratch (DRAM APs have no partition constraints):
            # idxw[16g + 2*r_loc + qq, pp, k, c] = idx16[row(qq), r_loc*128+c]
            # row = (2pp+qq)*32 + 2k+dd;  groups g<4 take dd=0, g>=4 dd=1
            nc.sync.dma_start(out=d_scr[:], in_=idx16[:])
            srcv = d_scr[:].rearrange("(a p) (r c) -> a p r c", a=4, r=8)
            for pp in range(2):
                for k in range(9):
                    for dd in range(2):
                        # src element order (r, qq, c) = dst partition
                        # s = 2r + qq ascending, free c
                        src = srcv[2 * pp: 2 * pp + 2, 2 * k + dd, :, :] \
                            .rearrange("q r c -> r q c")
                        for g in range(4):
                            nc.sync.dma_start(
                                out=idxw[dd * 64 + g * 16:
                                         dd * 64 + g * 16 + 16, pp, k, 0:128],
                                in_=src)
                    # second half: y1 = y0 + FC
                    nc.vector.tensor_scalar(idxw[:, pp, k, 128:256],
                                            idxw[:, pp, k, 0:128],
                                            FC, None, AL.add)

        # ================= Phase C: gather + Hadamard + einsum ==============
        with tc.tile_pool(name="psum_wb", bufs=2, space="PSUM") as psum_wb, \
             tc.tile_pool(name="psum_out", bufs=1, space="PSUM") as psum_out, \
             tc.tile_pool(name="gb", bufs=2) as gb_pool, \
             tc.tile_pool(name="wbs", bufs=2) as wbs_pool, \
             tc.tile_pool(name="rts", bufs=2) as rt_pool, \
             tc.tile_pool(name="outs", bufs=2) as out_pool:
            for pp in range(2):
                pos = [psum_out.tile([P, NCHUNK], F32, name=f"pos{pp}_{qq}",
                                     tag=f"pos{qq}") for qq in range(2)]
                for k in range(9):
                    g = gb_pool.tile([P, 4096, 2], F16)
                    nc.gpsimd.ap_gather(
                        g[:], fd[:, pp * PASSW: pp * PASSW + INW, :],
                        idxw[:, pp, k, :], channels=P,
                        num_elems=INW, d=2, num_idxs=4096)
                    # wbs layout: (yc, c, r_loc, qq, xc)
                    wbs = wbs_pool.tile([P, 2, 128, 8, 2, 2], F16)
                    for yc in range(2):
                        for xc in range(2):
                            for qq in range(2):
                                rowp = (2 * pp + qq) * 32
                                wb = psum_wb.tile([P, NCHUNK], F32, tag="wb")
                                # rhs in (c, r_loc) order -> psum col c*8+r
                                rhs = cw[rowp: rowp + 18, yc, :, xc] \
                                    .rearrange("p (r c) -> p c r", r=8)
                                for h5 in range(2):
                                    nc.tensor.matmul(
                                        wb[:, h5 * 512:(h5 + 1) * 512],
                                        esel[rowp: rowp + 18, k, :],
                                        rhs[:, h5 * 64:(h5 + 1) * 64, :],
                                        start=True, stop=True,
                                        tile_position=(rowp, 0))
                                nc.scalar.copy(
                                    wbs[:, yc, :, :, qq, xc],
                                    wb[:].rearrange("p (c r) -> p c r", c=128))
                    rt = rt_pool.tile([P, 2, 128, 8, 2, 2], F16)
                    for yc in range(2):
                        nc.vector.tensor_tensor(
                            rt[:, yc].rearrange("p b c d e -> p (b c d e)"),
                            wbs[:, yc].rearrange("p b c d e -> p (b c d e)"),
                            g[:, yc * 2048:(yc + 1) * 2048, :]
                            .rearrange("p a b -> p (a b)"),
                            AL.mult)
                    for yc in range(2):
                        for xc in range(2):
                            for qq in range(2):
                                rhs = rt[:, yc, :, :, qq, xc]
                                for h5 in range(2):
                                    nc.tensor.matmul(
                                        pos[qq][:, h5 * 512:(h5 + 1) * 512],
                                        ldcn[:, k, :],
                                        rhs[:, h5 * 64:(h5 + 1) * 64, :],
                                        start=(k == 0 and yc == 0
                                               and xc == 0),
                                        stop=(k == 8 and yc == 1
                                              and xc == 1))
                ob = out_pool.tile([P, 2, NCHUNK], F32)
                for qq in range(2):
                    nc.scalar.activation(ob[:, qq, :], pos[qq][:], AF.Relu,
                                         bias=beta3, scale=inv3)
                nc.sync.dma_start(out=d_out[:, pp, :, :], in_=ob[:])

    nc.compile()
    return nc


_CACHE = {}


def _prep(inputs):
    f = {k: _f(v) for k, v in inputs.items()}
    inv1 = f['g1'] / np.sqrt(f['v1'] + EPS)
    beta1 = f['b1'] - f['m1'] * inv1
    inv2 = f['g2'] / np.sqrt(f['v2'] + EPS)
    beta2 = f['b2'] - f['m2'] * inv2
    invd = f['gd'] / np.sqrt(f['vd'] + EPS)
    betad = f['bd'] - f['md'] * invd
    inv3 = f['g3'] / np.sqrt(f['v3'] + EPS)
    beta3 = f['b3'] - f['m3'] * inv3

    lhsT1 = np.transpose(f['w1'], (1, 2, 3, 0)).reshape(Ci, 9, P)
    lhsT2 = np.transpose(f['w2'], (1, 2, 3, 0)).reshape(P, 9, P)
    wd = f['wd'][:, :, 0, 0] * (invd / inv2)[:, None]
    lhsT_sc = np.ascontiguousarray(wd.T)

    # offset conv rows: quant*18 + k*2 + d  <-  orig quant*18 + d*9 + k
    perm = np.zeros(54, dtype=np.int64)
    for quant in range(3):
        for kk in range(9):
            for dd in range(2):
                perm[quant * 18 + kk * 2 + dd] = quant * 18 + dd * 9 + kk
    ow = f['off_w'][perm]
    obias = f['off_b'][perm]
    lhsT_off = np.transpose(ow, (1, 2, 3, 0)).reshape(P, 9, 54)

    wr = f['dcn_w'].reshape(Co, DG, Cg, 9)
    lhsT_dcn = np.transpose(wr, (1, 2, 3, 0)).reshape(P, 9, Co)

    esel = np.zeros((P, 9, P), dtype=np.float32)
    for s in range(4):
        for kk in range(9):
            for dd in range(2):
                esel[32 * s + 2 * kk + dd, kk, dd * 64:(dd + 1) * 64] = 1.0

    cst = np.zeros((P, 8), dtype=np.float32)
    cst[:, 0], cst[:, 1] = inv1, beta1
    cst[:, 2], cst[:, 3] = inv2, beta2 + betad
    cst[:, 4], cst[:, 5] = inv3, beta3 + inv3 * f['dcn_b']

    bias_q = np.zeros((P, 3), dtype=np.float32)
    for pq in range(4):
        for kk in range(9):
            for dd in range(2):
                r = pq * 32 + kk * 2 + dd
                for quant in range(3):
                    bias_q[r, quant] = obias[quant * 18 + kk * 2 + dd]

    # coordinate base maps (pass-relative row base)
    base_y = np.zeros((P, NCHUNK), dtype=np.float32)
    base_x = np.zeros((P, NCHUNK), dtype=np.float32)
    px = np.arange(NCHUNK)
    for p in range(P):
        pq, t = p // 32, p % 32
        if t >= 18:
            continue
        kk, dd = t // 2, t % 2
        ky, kx = kk // 3, kk % 3
        base_y[p] = 8 * (pq % 2) + px // 128 + ky + 2
        base_x[p] = px % 128 + kx + 2

    return dict(
        lhsT1=_h(lhsT1), lhsT2=_h(lhsT2), lhsT_sc=_h(lhsT_sc),
        lhsT_off=_h(lhsT_off), lhsT_dcn=_h(lhsT_dcn), e_sel=_h(esel),
        consts=_f(cst), bias_q=_f(bias_q), base_y=base_y, base_x=base_x,
        x=f['x'])


def kernel(**inputs):
    cfg = _prep(inputs)
    x = cfg.pop('x')
    B = x.shape[0]

    if 'nc' not in _CACHE:
        _CACHE['nc'] = build_nc()
    nc = _CACHE['nc']

    in_maps = []
    for cid in range(8):
        b, q = cid // 4, cid % 4
        h0 = 32 * q
        xp = np.zeros((Ci, XR, XC), dtype=np.float16)
        r_lo = 2 * h0 - 9
        s_lo, s_hi = max(r_lo, 0), min(2 * h0 + 72, 256)
        xp[:, s_lo - r_lo: s_hi - r_lo, 1:257] = \
            x[b, :, s_lo:s_hi, :].astype(np.float16)
        rm1 = np.zeros((P, F1R), dtype=np.float32)
        for f1 in range(F1R):
            rm1[:, f1] = 1.0 if 0 <= h0 - 4 + f1 < H else 0.0
        rmf = np.zeros((P, FR), dtype=np.float32)
        for f2 in range(FR):
            rmf[:, f2] = 1.0 if 0 <= h0 - 3 + f2 < H else 0.0
        m = dict(cfg)
        m['x_shard'] = np.ascontiguousarray(xp)
        m['rowmask1'] = rm1
        m['rowmaskF'] = rmf
        in_maps.append(m)

    res = run_bass_kernel_spmd(nc, in_maps, core_ids=list(range(8)))
    out = np.zeros((B, Co, H, W), dtype=np.float32)
    for cid in range(8):
        b, q = cid // 4, cid % 4
        o = res.results[cid]['out']            # [P, pp, qq, m=c*8+r_loc]
        o = o.reshape(P, 2, 2, 128, 8)         # [P, pp, qq, c, r]
        o = np.transpose(o, (0, 1, 2, 4, 3))   # [P, pp, qq, r, c]
        out[b, :, 32 * q:32 * q + 32, :] = o.reshape(P, 32, 128)
    return out
